# revision 17
# baseline (speedup 1.0000x reference)
"""Trainium2 Bass kernel for a 5-layer GENConv GNN (softmax aggregation) + dense head.

Strategy (8 NeuronCores, SPMD):
  - Host sorts edges by destination, pads nodes to 20480 (2560/core) and each
    128-node window's edge list to 2176 edges (17 tiles of 128).
  - Each core owns a contiguous 2560-node shard and the edges targeting it.
  - Node activations live transposed ([feat, node]) in SBUF; node linears are
    chained PE matmuls (bf16, fp32 PSUM) with a bias x mask-row matmul so
    padded nodes stay exactly zero.
  - Per layer: node linear -> bf16 row-major copy (DMA transpose) -> AllGather
    -> per-window dma_gather of source rows -> edge linear on PE ->
    msg = relu(x_src + e) -> softmax aggregation as indicator matmuls
    (sum(exp), sum(msg*exp) in one PSUM accumulation; segment-max skipped,
    equivalent since messages are O(1); numerically validated) -> per-node
    divide -> MLP with training-mode BatchNorm (stat sums AllReduced).
  - Head: pooling via indicator matmuls, AllReduce, replicated dense chain,
    log_softmax on-chip.
"""

import numpy as np
import ml_dtypes

import concourse.bacc as bacc
import concourse.bass as bass
import concourse.mybir as mybir
from concourse.bass_utils import run_bass_kernel_spmd
from concourse.library_config import mlp as _mlp_lib
from concourse.vector_clock import ScopedClock
import concourse.tile as tile

bf16 = ml_dtypes.bfloat16
dt = mybir.dt
AF = mybir.ActivationFunctionType
ALU = mybir.AluOpType

NCORES = 8
EPS_DEN = 1e-16
BN_EPS = 1e-5

# (c_in, c_out, c_pad_for_gather, has_ws)
LAYERS = [
    (128, 64, 128, True),
    (64, 64, 128, False),
    (64, 128, 128, True),
    (128, 256, 256, True),
    (256, 512, 512, True),
]
IN_DIM, OUT_DIM, N_GRAPHS = 128, 10, 50


class Cfg:
    def __init__(self, n_nodes, n_edges, shard, tiles_w, chunk_tiles=(6, 6, 5)):
        self.n_nodes = n_nodes
        self.shard = shard
        self.np_total = shard * NCORES
        assert self.np_total >= n_nodes
        self.windows = shard // 128
        self.tiles_w = tiles_w
        self.epw = tiles_w * 128
        self.epc = self.epw * self.windows
        self.n_edges = n_edges
        self.chunks = [c for c in chunk_tiles if c > 0]
        assert sum(self.chunks) == tiles_w
        self.nslice = 512
        assert shard % self.nslice == 0


FULL = Cfg(n_nodes=20000, n_edges=320000, shard=2560, tiles_w=17)


def _wrap_idx(idx_flat):
    n = idx_flat.shape[0]
    assert n % 16 == 0
    return idx_flat.reshape(n // 16, 16).T.copy()


def _chunk_w(w_mat, b_vec):
    """[ci, co] weight + [co] bias -> ([128, nk, co] zero-padded chunks, [1, co])."""
    ci, co = w_mat.shape
    nk = (ci + 127) // 128
    out = np.zeros((128, nk, co), np.float32)
    for k in range(nk):
        rows = w_mat[k * 128:(k + 1) * 128]
        out[:rows.shape[0], k] = rows
    return out.astype(bf16), b_vec.reshape(1, co).astype(bf16)


def host_prep(cfg, x, edge_attr, params, edge_index, batch):
    f32 = np.float32
    src = np.asarray(edge_index[0]).astype(np.int64)
    dst = np.asarray(edge_index[1]).astype(np.int64)
    x = np.asarray(x, f32)
    edge_attr = np.asarray(edge_attr, f32)
    batch = np.asarray(batch).astype(np.int64)

    order = np.argsort(dst, kind="stable")
    s_src, s_dst, s_ea = src[order], dst[order], edge_attr[order]

    n_pad, shard = cfg.np_total, cfg.shard
    W, TW, EPW = cfg.windows, cfg.tiles_w, cfg.epw

    per_core = []
    win_of = s_dst // 128
    win_starts = np.searchsorted(win_of, np.arange(n_pad // 128 + 1))
    for d in range(NCORES):
        gidx = np.zeros((W, 128, EPW // 16), np.int16)
        ind = np.zeros((W, 128, TW, 128), bf16)   # [w, p(edge-in-tile), t, dst-local]
        eaT = np.zeros((17, cfg.epc), bf16)
        for w in range(W):
            gw = d * W + w
            lo, hi = (win_starts[gw], win_starts[gw + 1]) if gw < n_pad // 128 else (0, 0)
            ne = hi - lo
            assert ne <= EPW, f"window {gw}: {ne} edges > {EPW}"
            idx_full = np.zeros(EPW, np.int64)
            idx_full[:ne] = s_src[lo:hi]
            base = colbase = 0
            for ct in cfg.chunks:
                ce = ct * 128
                gidx[w, :16, colbase:colbase + ce // 16] = _wrap_idx(idx_full[base:base + ce])
                base += ce
                colbase += ce // 16
            gidx[w] = np.tile(gidx[w, :16], (8, 1))
            edst_loc = (s_dst[lo:hi] - gw * 128).astype(np.int64)
            t_idx = np.arange(ne) // 128
            p_idx = np.arange(ne) % 128
            ind[w, p_idx, t_idx, edst_loc] = bf16(1.0)
            ea_w = np.zeros((EPW, 16), f32)
            ea_w[:ne] = s_ea[lo:hi]
            eaT[:16, w * EPW:(w + 1) * EPW] = ea_w.T.astype(bf16)
            ones = np.zeros(EPW, f32)
            ones[:ne] = 1.0
            eaT[16, w * EPW:(w + 1) * EPW] = ones.astype(bf16)
        per_core.append(dict(gidx=gidx, ind=ind, ea_t=eaT))

    x_pad = np.zeros((n_pad, IN_DIM), f32)
    x_pad[:cfg.n_nodes] = x
    for d in range(NCORES):
        blk = x_pad[d * shard:(d + 1) * shard]
        per_core[d]["xT0"] = np.ascontiguousarray(blk.T).astype(bf16)
        m = np.zeros((1, shard), f32)
        n_real = min(max(cfg.n_nodes - d * shard, 0), shard)
        m[0, :n_real] = 1.0
        per_core[d]["mask"] = m.astype(bf16)
        per_core[d]["mask128"] = np.tile(m, (128, 1)).astype(bf16)

    batch_pad = np.full(n_pad, -1, np.int64)
    batch_pad[:cfg.n_nodes] = batch
    cnt = np.bincount(batch, minlength=N_GRAPHS).astype(f32)
    inv_cnt = (1.0 / np.maximum(cnt, 1.0)).astype(f32)
    for d in range(NCORES):
        pind = np.zeros((128, W, N_GRAPHS), bf16)   # p-major
        bb = batch_pad[d * shard:(d + 1) * shard].reshape(W, 128)
        for w in range(W):
            valid = bb[w] >= 0
            pind[np.arange(128)[valid], w, bb[w][valid]] = bf16(1.0)
        per_core[d]["pool_ind"] = pind
        per_core[d]["inv_cnt"] = np.tile(inv_cnt[None, :], (128, 1))

    shared = {}
    for li, (ci, c, cpad, has_ws) in enumerate(LAYERS):
        p = params[f"conv{li + 1}"]
        if has_ws:
            shared[f"ws{li}"], shared[f"bs{li}"] = _chunk_w(
                np.asarray(p["Ws"], f32), np.asarray(p["bs"], f32))
        wea = np.zeros((17, c), f32)
        wea[:16] = np.asarray(p["We"], f32)
        wea[16] = np.asarray(p["be"], f32)
        shared[f"we{li}"] = wea.astype(bf16)
        shared[f"w1_{li}"], shared[f"b1_{li}"] = _chunk_w(
            np.asarray(p["W1"], f32), np.asarray(p["b1"], f32))
        shared[f"w2_{li}"], shared[f"b2_{li}"] = _chunk_w(
            np.asarray(p["W2"], f32), np.asarray(p["b2"], f32))
        nj2 = (2 * c) // 128
        shared[f"g1_{li}"] = np.asarray(p["g1"], f32).reshape(nj2, 128).T.copy()
        shared[f"be1_{li}"] = np.asarray(p["be1"], f32).reshape(nj2, 128).T.copy()

    for nm in ["dense1", "dense2", "dense3"]:
        wp = params[nm]
        shared[f"{nm}_w"], shared[f"{nm}_b"] = _chunk_w(
            np.asarray(wp["W"], f32), np.asarray(wp["b"], f32))

    shared["ident"] = np.eye(128, dtype=f32)

    in_maps = []
    for d in range(NCORES):
        m = dict(per_core[d])
        m.update(shared)
        in_maps.append(m)
    return in_maps


class TileContextP(tile.TileContext):
    """Kernel-tail drain emits one sync wait per instruction (walrus limit)."""

    def _drain_and_barrier(self, tick_clock, wait_clock):
        carrier = self.nc.sync.nop(nofuse=True)
        wait_clock.add_sem_waits(carrier.ins, ScopedClock({None: tick_clock.global_clock}))
        si = carrier.ins.sync_info
        waits = list(si.on_wait) if si and si.on_wait else []
        if len(waits) > 1:
            si.on_wait.clear()
            si.on_wait.append(waits[0])
            for w in waits[1:]:
                n2 = self.nc.sync.nop(nofuse=True)
                si2 = n2.ins.sync_info
                if si2 is None:
                    n2.ins.sync_info = si2 = mybir.SyncInfo(on_wait=[], on_update=[])
                si2.on_wait.append(w)
        self.nc.sync.drain()
        self.nc.all_engine_barrier()
        assert self.sems is not None
        popped = self.nc._tile_sem_poison_stack.pop()
        assert popped is self._sem_poison
        self.nc.clear_and_free_semaphores(list(self.sems.allocated().values()))
        self.nc.all_engine_barrier()



_TN = [0]


def _t(pool, shape, dtp, tag):
    _TN[0] += 1
    return pool.tile(shape, dtp, tag=tag, name=f"{tag}_{_TN[0]}")

def build_nc(cfg):
    nc = bacc.Bacc(None, target_bir_lowering=False, num_devices=NCORES)
    W, TW, EPW, shard = cfg.windows, cfg.tiles_w, cfg.epw, cfg.shard
    NS = cfg.nslice
    nsl = shard // NS
    rg = [list(range(NCORES))]

    def din(name, shape, dtp=dt.bfloat16):
        return nc.dram_tensor(name, shape, dtp, kind="ExternalInput")

    xT0 = din("xT0", [IN_DIM, shard])
    mask = din("mask", [1, shard])
    mask128 = din("mask128", [128, shard])
    gidx_d = din("gidx", [W, 128, EPW // 16], dt.int16)
    ind_d = din("ind", [W, 128, TW, 128])
    ea_d = din("ea_t", [17, cfg.epc])
    pool_d = din("pool_ind", [128, W, N_GRAPHS])
    invc_d = din("inv_cnt", [128, N_GRAPHS], dt.float32)
    ident_d = din("ident", [128, 128], dt.float32)
    wts = {}
    for li, (ci, c, cpad, has_ws) in enumerate(LAYERS):
        nk = (ci + 127) // 128
        if has_ws:
            wts[f"ws{li}"] = din(f"ws{li}", [128, nk, c])
            wts[f"bs{li}"] = din(f"bs{li}", [1, c])
        wts[f"we{li}"] = din(f"we{li}", [17, c])
        wts[f"w1_{li}"] = din(f"w1_{li}", [128, max(c // 128, 1), 2 * c])
        wts[f"b1_{li}"] = din(f"b1_{li}", [1, 2 * c])
        wts[f"w2_{li}"] = din(f"w2_{li}", [128, (2 * c) // 128, c])
        wts[f"b2_{li}"] = din(f"b2_{li}", [1, c])
        wts[f"g1_{li}"] = din(f"g1_{li}", [128, (2 * c) // 128], dt.float32)
        wts[f"be1_{li}"] = din(f"be1_{li}", [128, (2 * c) // 128], dt.float32)
    for nm, (ci, co) in [("dense1", (512, 512)), ("dense2", (512, 256)), ("dense3", (256, 10))]:
        wts[f"{nm}_w"] = din(f"{nm}_w", [128, ci // 128, co])
        wts[f"{nm}_b"] = din(f"{nm}_b", [1, co])

    out_d = nc.dram_tensor("out", [N_GRAPHS, OUT_DIM], dt.float32, kind="ExternalOutput")
    import os as _os3
    DBG = _os3.environ.get("GNN_DEBUG", "0") == "1"
    dbg_h = {}
    if DBG:
        for li, (ci, c, cpad, has_ws) in enumerate(LAYERS):
            dbg_h[li] = nc.dram_tensor(f"dbg_h{li}", [min(c, 128), max(c // 128, 1), shard],
                                       dt.bfloat16, kind="ExternalOutput")
            dbg_h[(li, "agg")] = nc.dram_tensor(
                f"dbg_agg{li}", [min(c, 128), max(c // 128, 1), shard],
                dt.bfloat16, kind="ExternalOutput")
            dbg_h[(li, "u")] = nc.dram_tensor(
                f"dbg_u{li}", [128, (2 * c) // 128, shard], dt.bfloat16, kind="ExternalOutput")
            dbg_h[(li, "st")] = nc.dram_tensor(
                f"dbg_st{li}", [128, ((2 * c) // 128) * 2], dt.float32, kind="ExternalOutput")

    ag_in, xt_full, st_in, st_out = {}, {}, {}, {}
    for li, (ci, c, cpad, has_ws) in enumerate(LAYERS):
        ag_in[li] = nc.dram_tensor(f"ag_in{li}", [shard, cpad], dt.bfloat16)
        xt_full[li] = nc.dram_tensor(f"xt_full{li}", [cfg.np_total, cpad], dt.bfloat16,
                                     addr_space="Shared")
        nst = ((2 * c) // 128) * 2
        st_in[li] = nc.dram_tensor(f"st_in{li}", [128, nst], dt.float32)
        st_out[li] = nc.dram_tensor(f"st_out{li}", [128, nst], dt.float32, addr_space="Shared")
    pool_in = nc.dram_tensor("pool_in", [128, 4 * N_GRAPHS], dt.float32)
    pool_out = nc.dram_tensor("pool_out", [128, 4 * N_GRAPHS], dt.float32, addr_space="Shared")

    inv_n = 1.0 / float(cfg.n_nodes)

    with TileContextP(nc) as tc:
        nc.gpsimd.load_library(_mlp_lib)
        with (
            tc.tile_pool(name="const", bufs=1) as cpool,
            tc.tile_pool(name="acts", bufs=1) as apool,
            tc.tile_pool(name="win", bufs=2) as wpool,
            tc.tile_pool(name="win1", bufs=1) as w1pool,
            tc.tile_pool(name="wt", bufs=2) as wtp,
            tc.tile_pool(name="small", bufs=2) as spool,
            tc.tile_pool(name="ps", bufs=2, space="PSUM") as psp,
            tc.tile_pool(name="ps1", bufs=1, space="PSUM") as ps1p,
        ):
            ident = _t(cpool, [128, 128], dt.float32, "ident")
            nc.sync.dma_start(ident[:], ident_d[:])
            mask_t = _t(cpool, [1, shard], dt.bfloat16, "mask")
            nc.sync.dma_start(mask_t[:], mask[:])
            mask128_t = _t(cpool, [128, shard], dt.bfloat16, "mask128")
            nc.sync.dma_start(mask128_t[:], mask128[:])
            invc_t = _t(cpool, [128, N_GRAPHS], dt.float32, "invc")
            nc.sync.dma_start(invc_t[:], invc_d[:])

            def new_xT(cdim, tag):
                nj_ = max(cdim // 128, 1)
                return _t(apool, [min(cdim, 128), nj_, shard], dt.bfloat16, tag)

            xT = new_xT(IN_DIM, "xT_a")
            nc.sync.dma_start(xT[:, 0, :], xT0[:])

            def matmul_chain(out_ps, w_tile, colsl, b_tile, rhs_tile, rhs_ci, n0, n1):
                """out_ps[M, n1-n0] = sum_k w[kchunk, cols].T @ rhs[kchunk, n0:n1]
                + b[cols].T @ mask[n0:n1]"""
                nj_ = max(rhs_ci // 128, 1)
                kc = min(rhs_ci, 128)
                c0, c1 = colsl
                for k in range(nj_):
                    nc.tensor.matmul(out_ps, w_tile[:kc, k, c0:c1], rhs_tile[:kc, k, n0:n1],
                                     start=(k == 0), stop=False)
                nc.tensor.matmul(out_ps, b_tile[:, c0:c1], mask_t[:, n0:n1],
                                 start=False, stop=True)

            for li, (ci, c, cpad, has_ws) in enumerate(LAYERS):
                nj = max(c // 128, 1)
                pdim = min(c, 128)
                nj2 = (2 * c) // 128
                nk_in = max(ci // 128, 1)

                # ---------- Phase A: xt = x @ Ws + bs (or alias)
                if has_ws:
                    w_ws = _t(wtp, [128, nk_in, c], dt.bfloat16, "wbig")
                    nc.sync.dma_start(w_ws[:], wts[f"ws{li}"][:])
                    b_ws = _t(wtp, [1, c], dt.bfloat16, "wb")
                    nc.sync.dma_start(b_ws[:], wts[f"bs{li}"][:])
                    xtT = _t(apool, [pdim, nj, shard], dt.bfloat16, "xtT")
                    for j in range(nj):
                        for n in range(nsl):
                            ps = _t(psp, [pdim, NS], dt.float32, "mmps")
                            matmul_chain(ps[:], w_ws, (j * 128, j * 128 + pdim), b_ws,
                                         xT, ci, n * NS, (n + 1) * NS)
                            nc.scalar.copy(xtT[:, j, n * NS:(n + 1) * NS], ps[:])
                else:
                    xtT = xT

                # ---------- Phase A': row-major bf16 + AllGather
                xt_row = _t(w1pool, [128, W, cpad], dt.bfloat16, "msg")
                if cpad != c:
                    nc.vector.memset(xt_row[:, :, c:cpad], 0.0)
                for j in range(nj):
                    for t in range(W):
                        nc.sync.dma_start_transpose(
                            xt_row[:, t, j * 128:j * 128 + pdim],
                            xtT[:pdim, j, t * 128:(t + 1) * 128],
                        )
                nc.sync.dma_start(
                    ag_in[li][:].rearrange("(t p) c -> p t c", p=128),
                    xt_row[:],
                )
                nc.gpsimd.collective_compute(
                    "AllGather", ALU.bypass, replica_groups=rg,
                    ins=[ag_in[li][:].opt()], outs=[xt_full[li][:].opt()],
                )

                # ---------- Phase B: edge stage
                w_we = _t(wtp, [17, c], dt.bfloat16, "wwe")
                nc.sync.dma_start(w_we[:], wts[f"we{li}"][:])
                maxct = max(cfg.chunks)
                for w in range(W):
                    msg = _t(w1pool, [128, TW, c], dt.bfloat16, "msgb")
                    ext = _t(w1pool, [128, TW, c], dt.bfloat16, "msg")
                    seg_e = _t(ps1p, [128, c], dt.float32, "sege")
                    seg_p = _t(ps1p, [128, c], dt.float32, "segp")
                    gt = 0
                    colbase = 0
                    for ct in cfg.chunks:
                        nidx = ct * 128
                        gx = _t(wpool, [128, maxct, cpad], dt.bfloat16, "gx")
                        gi = _t(wpool, [128, EPW // 16], dt.int16, "gi")
                        nc.sync.dma_start(gi[:, colbase:colbase + nidx // 16],
                                          gidx_d[w, :, colbase:colbase + nidx // 16])
                        nc.gpsimd.dma_gather(
                            gx[:, :ct, :], xt_full[li][:],
                            gi[:, colbase:colbase + nidx // 16],
                            nidx, nidx, cpad,
                        )
                        ea_w = _t(wpool, [17, maxct * 128], dt.bfloat16, "eaw")
                        nc.sync.dma_start(
                            ea_w[:, :nidx],
                            ea_d[:, w * EPW + gt * 128: w * EPW + gt * 128 + nidx],
                        )
                        for t in range(ct):
                            eps = _t(psp, [128, c], dt.float32, "eps")
                            nc.tensor.matmul(eps[:], ea_w[:, t * 128:(t + 1) * 128],
                                             w_we[:], start=True, stop=True)
                            nc.vector.tensor_add(msg[:, gt + t, :], gx[:, t, :c], eps[:])
                        gt += ct
                        colbase += nidx // 16
                    nc.vector.tensor_scalar_max(msg[:], msg[:], 0.0)
                    nc.scalar.activation(ext[:], msg[:], AF.Exp)
                    import os as _os
                    if _os.environ.get("GNN_INPLACE_P", "1") == "1":
                        nc.vector.tensor_mul(msg[:], msg[:], ext[:])  # p in place
                    else:
                        pt_ = _t(w1pool, [128, TW, c], dt.bfloat16, "pt")
                        nc.vector.tensor_mul(pt_[:], msg[:], ext[:])
                        msg = pt_
                    ind_w = _t(w1pool, [128, TW, 128], dt.bfloat16, "indw")
                    nc.sync.dma_start(ind_w[:], ind_d[w, :, :, :])
                    for t in range(TW):
                        for hh in range(0, c, 512):
                            he = min(hh + 512, c)
                            nc.tensor.matmul(
                                seg_e[:, hh:he], ind_w[:, t, :], ext[:, t, hh:he],
                                start=(t == 0), stop=(t == TW - 1),
                                skip_group_check=(hh > 0),
                            )
                            nc.tensor.matmul(
                                seg_p[:, hh:he], ind_w[:, t, :], msg[:, t, hh:he],
                                start=(t == 0), stop=(t == TW - 1),
                                skip_group_check=(hh > 0),
                            )
                    dwin = _t(spool, [128, c], dt.float32, "dwin")
                    nc.vector.tensor_scalar_add(dwin[:], seg_e[:], EPS_DEN)
                    rec = _t(spool, [128, c], dt.float32, "rec")
                    nc.vector.reciprocal_approx_fast(rec[:], dwin[:])
                    aggr = _t(spool, [128, c], dt.float32, "aggr")
                    nc.vector.tensor_mul(aggr[:], rec[:], seg_p[:])
                    for j in range(nj):
                        tps = _t(psp, [128, 128], dt.float32, "trps")
                        nc.tensor.transpose(tps[:pdim, :], aggr[:, j * 128:j * 128 + pdim],
                                            ident[:])
                        nc.vector.tensor_add(
                            xtT[:pdim, j, w * 128:(w + 1) * 128],
                            xtT[:pdim, j, w * 128:(w + 1) * 128],
                            tps[:pdim, :],
                        )

                if DBG:
                    nc.sync.dma_start(dbg_h[(li, "agg")][:], xtT[:pdim, :nj, :])

                # ---------- Phase C: u = h_mid @ W1 + b1; BN; relu; W2
                w_w1 = _t(wtp, [128, nj, 2 * c], dt.bfloat16, "wbig")
                nc.sync.dma_start(w_w1[:], wts[f"w1_{li}"][:])
                b_w1 = _t(wtp, [1, 2 * c], dt.bfloat16, "wb")
                nc.sync.dma_start(b_w1[:], wts[f"b1_{li}"][:])
                u = _t(apool, [128, nj2, shard], dt.bfloat16, "u")
                statsS = _t(spool, [128, nj2, nsl], dt.float32, "statsS")
                stats2 = _t(spool, [128, nj2, nsl], dt.float32, "stats2")
                junk = _t(apool, [128, NS], dt.bfloat16, "junk")
                for j2 in range(nj2):
                    for n in range(nsl):
                        ps = _t(psp, [128, NS], dt.float32, "mmps")
                        matmul_chain(ps[:], w_w1, (j2 * 128, (j2 + 1) * 128), b_w1,
                                     xtT, c, n * NS, (n + 1) * NS)
                        nc.scalar.copy(u[:, j2, n * NS:(n + 1) * NS], ps[:])
                        nc.vector.reduce_sum(statsS[:, j2, n:n + 1], ps[:],
                                             axis=mybir.AxisListType.X)
                        nc.scalar.activation(junk[:], ps[:], AF.Square,
                                             accum_out=stats2[:, j2, n:n + 1])
                stats = _t(spool, [128, nj2, 2], dt.float32, "stats")
                nc.vector.reduce_sum(stats[:, :, 0:1], statsS[:],
                                     axis=mybir.AxisListType.X)
                nc.vector.reduce_sum(stats[:, :, 1:2], stats2[:],
                                     axis=mybir.AxisListType.X)
                if DBG:
                    nc.sync.dma_start(dbg_h[(li, "u")][:], u[:, :nj2, :])
                    nc.sync.dma_start(dbg_h[(li, "st")][:], stats[:].rearrange("p a b -> p (a b)"))
                nc.sync.dma_start(st_in[li][:], stats[:].rearrange("p a b -> p (a b)"))
                nc.gpsimd.collective_compute(
                    "AllReduce", ALU.add, replica_groups=rg,
                    ins=[st_in[li][:].opt()], outs=[st_out[li][:].opt()],
                )
                statr = _t(spool, [128, nj2, 2], dt.float32, "statr")
                nc.sync.dma_start(statr[:].rearrange("p a b -> p (a b)"), st_out[li][:])
                g1t = _t(spool, [128, nj2], dt.float32, "g1t")
                nc.sync.dma_start(g1t[:], wts[f"g1_{li}"][:])
                be1t = _t(spool, [128, nj2], dt.float32, "be1t")
                nc.sync.dma_start(be1t[:], wts[f"be1_{li}"][:])
                mu = _t(spool, [128, nj2], dt.float32, "mu")
                nc.vector.tensor_scalar_mul(mu[:], statr[:, :, 0], inv_n)
                msq = _t(spool, [128, nj2], dt.float32, "msq")
                nc.vector.tensor_scalar_mul(msq[:], statr[:, :, 1], inv_n)
                var = _t(spool, [128, nj2], dt.float32, "var")
                nc.vector.scalar_tensor_tensor(var[:], mu[:], -1.0, mu[:],
                                               op0=ALU.mult, op1=ALU.mult)
                nc.vector.tensor_add(var[:], var[:], msq[:])
                nc.vector.tensor_scalar_max(var[:], var[:], 0.0)
                vr = _t(spool, [128, nj2], dt.float32, "vr")
                nc.vector.tensor_scalar_add(vr[:], var[:], BN_EPS)
                rvr = _t(spool, [128, nj2], dt.float32, "rvr")
                nc.vector.reciprocal(rvr[:], vr[:])
                inv_std = _t(spool, [128, nj2], dt.float32, "invs")
                nc.scalar.sqrt(inv_std[:], rvr[:])
                A_t = _t(spool, [128, nj2], dt.float32, "A_t")
                nc.vector.tensor_mul(A_t[:], g1t[:], inv_std[:])
                B_t = _t(spool, [128, nj2], dt.float32, "B_t")
                nc.vector.scalar_tensor_tensor(B_t[:], mu[:], -1.0, A_t[:],
                                               op0=ALU.mult, op1=ALU.mult)
                nc.vector.tensor_add(B_t[:], B_t[:], be1t[:])
                for j2 in range(nj2):
                    nc.scalar.activation(u[:, j2, :], u[:, j2, :], AF.Relu,
                                         bias=B_t[:, j2:j2 + 1], scale=A_t[:, j2:j2 + 1])
                    nc.vector.tensor_mul(u[:, j2, :], u[:, j2, :], mask128_t[:])
                w_w2 = _t(wtp, [128, nj2, c], dt.bfloat16, "wbig")
                nc.sync.dma_start(w_w2[:], wts[f"w2_{li}"][:])
                b_w2 = _t(wtp, [1, c], dt.bfloat16, "wb")
                nc.sync.dma_start(b_w2[:], wts[f"b2_{li}"][:])
                xT_next = new_xT(c, "xT_b" if li % 2 == 0 else "xT_a")
                for j in range(nj):
                    for n in range(nsl):
                        ps = _t(psp, [pdim, NS], dt.float32, "mmps")
                        matmul_chain(ps[:], w_w2, (j * 128, j * 128 + pdim), b_w2,
                                     u, 2 * c, n * NS, (n + 1) * NS)
                        nc.scalar.activation(xT_next[:, j, n * NS:(n + 1) * NS],
                                             ps[:], AF.Relu)
                if DBG:
                    nc.sync.dma_start(dbg_h[li][:], xT_next[:pdim, :nj, :])
                xT = xT_next

            # ---------- pooling + head
            h_row = _t(w1pool, [128, W, 512], dt.bfloat16, "msg")
            for j in range(4):
                for t in range(W):
                    nc.sync.dma_start_transpose(
                        h_row[:, t, j * 128:(j + 1) * 128],
                        xT[:, j, t * 128:(t + 1) * 128],
                    )
            pind_t = _t(cpool, [128, W, N_GRAPHS], dt.bfloat16, "pind")
            nc.sync.dma_start(pind_t[:], pool_d[:])
            gsb = _t(spool, [128, 4, N_GRAPHS], dt.float32, "gsb")
            for j in range(4):
                gps = _t(ps1p, [128, N_GRAPHS], dt.float32, "sege")
                for t in range(W):
                    nc.tensor.matmul(gps[:], h_row[:, t, j * 128:(j + 1) * 128],
                                     pind_t[:, t, :], start=(t == 0), stop=(t == W - 1))
                nc.vector.tensor_copy(gsb[:, j, :], gps[:])
            nc.sync.dma_start(pool_in[:], gsb[:].rearrange("p a b -> p (a b)"))
            nc.gpsimd.collective_compute(
                "AllReduce", ALU.add, replica_groups=rg,
                ins=[pool_in[:].opt()], outs=[pool_out[:].opt()],
            )
            gT = _t(spool, [128, 4, N_GRAPHS], dt.float32, "gT")
            nc.sync.dma_start(gT[:].rearrange("p a b -> p (a b)"), pool_out[:])
            gTb = _t(spool, [128, 4, N_GRAPHS], dt.bfloat16, "gTb")
            for j in range(4):
                nc.vector.tensor_mul(gTb[:, j, :], gT[:, j, :], invc_t[:])

            ones50 = mask_t[:, :N_GRAPHS]

            def dense_T(src_t, srcchunks, wname, co):
                wt_ = _t(wtp, [128, srcchunks, co], dt.bfloat16, "wbig")
                nc.sync.dma_start(wt_[:], wts[f"{wname}_w"][:])
                bt_ = _t(wtp, [1, co], dt.bfloat16, "wb")
                nc.sync.dma_start(bt_[:], wts[f"{wname}_b"][:])
                out_t = _t(spool, [128, co // 128, N_GRAPHS], dt.bfloat16, f"dT{co}")
                for j in range(co // 128):
                    ps = _t(psp, [128, N_GRAPHS], dt.float32, "mmps")
                    for k in range(srcchunks):
                        nc.tensor.matmul(ps[:], wt_[:, k, j * 128:(j + 1) * 128],
                                         src_t[:, k, :], start=(k == 0), stop=False)
                    nc.tensor.matmul(ps[:], bt_[:, j * 128:(j + 1) * 128],
                                     ones50, start=False, stop=True)
                    nc.vector.tensor_copy(out_t[:, j, :], ps[:])
                return out_t

            g1_t = dense_T(gTb, 4, "dense1", 512)
            g2_t = dense_T(g1_t, 4, "dense2", 256)
            w3t = _t(wtp, [128, 2, OUT_DIM], dt.bfloat16, "wbig")
            nc.sync.dma_start(w3t[:], wts["dense3_w"][:])
            b3t = _t(wtp, [1, OUT_DIM], dt.bfloat16, "wb")
            nc.sync.dma_start(b3t[:], wts["dense3_b"][:])
            o3 = _t(ps1p, [N_GRAPHS, OUT_DIM], dt.float32, "sege")
            for k in range(2):
                nc.tensor.matmul(o3[:], g2_t[:, k, :], w3t[:, k, :],
                                 start=(k == 0), stop=False)
            nc.tensor.matmul(o3[:], ones50, b3t[:], start=False, stop=True)
            mx = _t(spool, [N_GRAPHS, 1], dt.float32, "mx")
            nc.vector.reduce_max(mx[:], o3[:], axis=mybir.AxisListType.X)
            nmx = _t(spool, [N_GRAPHS, 1], dt.float32, "nmx")
            nc.vector.tensor_scalar_mul(nmx[:], mx[:], -1.0)
            ex3 = _t(spool, [N_GRAPHS, OUT_DIM], dt.float32, "ex3")
            nc.scalar.activation(ex3[:], o3[:], AF.Exp, bias=nmx[:])
            s3 = _t(spool, [N_GRAPHS, 1], dt.float32, "s3")
            nc.vector.reduce_sum(s3[:], ex3[:], axis=mybir.AxisListType.X)
            ls3 = _t(spool, [N_GRAPHS, 1], dt.float32, "ls3")
            nc.scalar.activation(ls3[:], s3[:], AF.Ln)
            off = _t(spool, [N_GRAPHS, 1], dt.float32, "off")
            nc.vector.scalar_tensor_tensor(off[:], ls3[:], -1.0, nmx[:],
                                           op0=ALU.mult, op1=ALU.add)
            fin = _t(spool, [N_GRAPHS, OUT_DIM], dt.float32, "fin")
            nc.scalar.activation(fin[:], o3[:], AF.Identity, bias=off[:])
            nc.sync.dma_start(out_d[:], fin[:])

    nc.compile()
    return nc


_CACHE = {}


def _get_nc(cfg):
    key = (cfg.n_nodes, cfg.shard, cfg.tiles_w)
    if key not in _CACHE:
        _CACHE[key] = build_nc(cfg)
    return _CACHE[key]


def run(cfg, x, edge_attr, params, edge_index, batch, trace=False):
    in_maps = host_prep(cfg, x, edge_attr, params, edge_index, batch)
    nc = _get_nc(cfg)
    return run_bass_kernel_spmd(nc, in_maps, core_ids=list(range(NCORES)), trace=trace)


def kernel(x, edge_attr, params, edge_index, batch):
    res = run(FULL, x, edge_attr, params, edge_index, batch)
    return res.results[0]["out"]


# revision 18
# speedup vs baseline: 1.0449x; 1.0449x over previous
"""Trainium2 Bass kernel for a 5-layer GENConv GNN (softmax aggregation) + dense head.

Strategy (8 NeuronCores, SPMD):
  - Host sorts edges by destination, pads nodes to 20480 (2560/core) and each
    128-node window's edge list to 2176 edges (17 tiles of 128).
  - Each core owns a contiguous 2560-node shard and the edges targeting it.
  - Node activations live transposed ([feat, node]) in SBUF; node linears are
    chained PE matmuls (bf16, fp32 PSUM) with a bias x mask-row matmul so
    padded nodes stay exactly zero.
  - Per layer: node linear -> bf16 row-major copy (DMA transpose) -> AllGather
    -> per-window dma_gather of source rows -> edge linear on PE ->
    msg = relu(x_src + e) -> softmax aggregation as indicator matmuls
    (sum(exp), sum(msg*exp) in one PSUM accumulation; segment-max skipped,
    equivalent since messages are O(1); numerically validated) -> per-node
    divide -> MLP with training-mode BatchNorm (stat sums AllReduced).
  - Head: pooling via indicator matmuls, AllReduce, replicated dense chain,
    log_softmax on-chip.
"""

import numpy as np
import ml_dtypes

import concourse.bacc as bacc
import concourse.bass as bass
import concourse.mybir as mybir
from concourse.bass_utils import run_bass_kernel_spmd
from concourse.library_config import mlp as _mlp_lib
from concourse.vector_clock import ScopedClock
import concourse.tile as tile

bf16 = ml_dtypes.bfloat16
dt = mybir.dt
AF = mybir.ActivationFunctionType
ALU = mybir.AluOpType

NCORES = 8
EPS_DEN = 1e-16
BN_EPS = 1e-5

# (c_in, c_out, c_pad_for_gather, has_ws)
LAYERS = [
    (128, 64, 128, True),
    (64, 64, 128, False),
    (64, 128, 128, True),
    (128, 256, 256, True),
    (256, 512, 512, True),
]
IN_DIM, OUT_DIM, N_GRAPHS = 128, 10, 50


class Cfg:
    def __init__(self, n_nodes, n_edges, shard, tiles_w, chunk_tiles=(6, 6, 5)):
        self.n_nodes = n_nodes
        self.shard = shard
        self.np_total = shard * NCORES
        assert self.np_total >= n_nodes
        self.windows = shard // 128
        self.tiles_w = tiles_w
        self.epw = tiles_w * 128
        self.epc = self.epw * self.windows
        self.n_edges = n_edges
        self.chunks = [c for c in chunk_tiles if c > 0]
        assert sum(self.chunks) == tiles_w
        self.nslice = 512
        assert shard % self.nslice == 0


FULL = Cfg(n_nodes=20000, n_edges=320000, shard=2560, tiles_w=17)


def _wrap_idx(idx_flat):
    n = idx_flat.shape[0]
    assert n % 16 == 0
    return idx_flat.reshape(n // 16, 16).T.copy()


def _chunk_w(w_mat, b_vec):
    """[ci, co] weight + [co] bias -> ([128, nk, co] zero-padded chunks, [1, co])."""
    ci, co = w_mat.shape
    nk = (ci + 127) // 128
    out = np.zeros((128, nk, co), np.float32)
    for k in range(nk):
        rows = w_mat[k * 128:(k + 1) * 128]
        out[:rows.shape[0], k] = rows
    return out.astype(bf16), b_vec.reshape(1, co).astype(bf16)


def host_prep(cfg, x, edge_attr, params, edge_index, batch):
    f32 = np.float32
    src = np.asarray(edge_index[0]).astype(np.int64)
    dst = np.asarray(edge_index[1]).astype(np.int64)
    x = np.asarray(x, f32)
    edge_attr = np.asarray(edge_attr, f32)
    batch = np.asarray(batch).astype(np.int64)

    order = np.argsort(dst, kind="stable")
    s_src, s_dst, s_ea = src[order], dst[order], edge_attr[order]

    n_pad, shard = cfg.np_total, cfg.shard
    W, TW, EPW = cfg.windows, cfg.tiles_w, cfg.epw

    per_core = []
    win_of = s_dst // 128
    win_starts = np.searchsorted(win_of, np.arange(n_pad // 128 + 1))
    for d in range(NCORES):
        gidx = np.zeros((W, 128, EPW // 16), np.int16)
        ind = np.zeros((W, 128, TW, 128), bf16)   # [w, p(edge-in-tile), t, dst-local]
        eaT = np.zeros((17, cfg.epc), bf16)
        for w in range(W):
            gw = d * W + w
            lo, hi = (win_starts[gw], win_starts[gw + 1]) if gw < n_pad // 128 else (0, 0)
            ne = hi - lo
            assert ne <= EPW, f"window {gw}: {ne} edges > {EPW}"
            idx_full = np.zeros(EPW, np.int64)
            idx_full[:ne] = s_src[lo:hi]
            base = colbase = 0
            for ct in cfg.chunks:
                ce = ct * 128
                gidx[w, :16, colbase:colbase + ce // 16] = _wrap_idx(idx_full[base:base + ce])
                base += ce
                colbase += ce // 16
            gidx[w] = np.tile(gidx[w, :16], (8, 1))
            edst_loc = (s_dst[lo:hi] - gw * 128).astype(np.int64)
            t_idx = np.arange(ne) // 128
            p_idx = np.arange(ne) % 128
            ind[w, p_idx, t_idx, edst_loc] = bf16(1.0)
            ea_w = np.zeros((EPW, 16), f32)
            ea_w[:ne] = s_ea[lo:hi]
            eaT[:16, w * EPW:(w + 1) * EPW] = ea_w.T.astype(bf16)
            ones = np.zeros(EPW, f32)
            ones[:ne] = 1.0
            eaT[16, w * EPW:(w + 1) * EPW] = ones.astype(bf16)
        per_core.append(dict(gidx=gidx, ind=ind, ea_t=eaT))

    x_pad = np.zeros((n_pad, IN_DIM), f32)
    x_pad[:cfg.n_nodes] = x
    for d in range(NCORES):
        blk = x_pad[d * shard:(d + 1) * shard]
        per_core[d]["xT0"] = np.ascontiguousarray(blk.T).astype(bf16)
        m = np.zeros((1, shard), f32)
        n_real = min(max(cfg.n_nodes - d * shard, 0), shard)
        m[0, :n_real] = 1.0
        per_core[d]["mask"] = m.astype(bf16)
        per_core[d]["mask128"] = np.tile(m, (128, 1)).astype(bf16)

    batch_pad = np.full(n_pad, -1, np.int64)
    batch_pad[:cfg.n_nodes] = batch
    cnt = np.bincount(batch, minlength=N_GRAPHS).astype(f32)
    inv_cnt = (1.0 / np.maximum(cnt, 1.0)).astype(f32)
    for d in range(NCORES):
        pind = np.zeros((128, W, N_GRAPHS), bf16)   # p-major
        bb = batch_pad[d * shard:(d + 1) * shard].reshape(W, 128)
        for w in range(W):
            valid = bb[w] >= 0
            pind[np.arange(128)[valid], w, bb[w][valid]] = bf16(1.0)
        per_core[d]["pool_ind"] = pind
        per_core[d]["inv_cnt"] = np.tile(inv_cnt[None, :], (128, 1))

    shared = {}
    for li, (ci, c, cpad, has_ws) in enumerate(LAYERS):
        p = params[f"conv{li + 1}"]
        if has_ws:
            shared[f"ws{li}"], shared[f"bs{li}"] = _chunk_w(
                np.asarray(p["Ws"], f32), np.asarray(p["bs"], f32))
        wea = np.zeros((17, c), f32)
        wea[:16] = np.asarray(p["We"], f32)
        wea[16] = np.asarray(p["be"], f32)
        shared[f"we{li}"] = wea.astype(bf16)
        shared[f"w1_{li}"], shared[f"b1_{li}"] = _chunk_w(
            np.asarray(p["W1"], f32), np.asarray(p["b1"], f32))
        shared[f"w2_{li}"], shared[f"b2_{li}"] = _chunk_w(
            np.asarray(p["W2"], f32), np.asarray(p["b2"], f32))
        nj2 = (2 * c) // 128
        shared[f"g1_{li}"] = np.asarray(p["g1"], f32).reshape(nj2, 128).T.copy()
        shared[f"be1_{li}"] = np.asarray(p["be1"], f32).reshape(nj2, 128).T.copy()

    for nm in ["dense1", "dense2", "dense3"]:
        wp = params[nm]
        shared[f"{nm}_w"], shared[f"{nm}_b"] = _chunk_w(
            np.asarray(wp["W"], f32), np.asarray(wp["b"], f32))

    shared["ident"] = np.eye(128, dtype=f32)

    in_maps = []
    for d in range(NCORES):
        m = dict(per_core[d])
        m.update(shared)
        in_maps.append(m)
    return in_maps


class TileContextP(tile.TileContext):
    """Kernel-tail drain emits one sync wait per instruction (walrus limit)."""

    def _drain_and_barrier(self, tick_clock, wait_clock):
        carrier = self.nc.sync.nop(nofuse=True)
        wait_clock.add_sem_waits(carrier.ins, ScopedClock({None: tick_clock.global_clock}))
        si = carrier.ins.sync_info
        waits = list(si.on_wait) if si and si.on_wait else []
        if len(waits) > 1:
            si.on_wait.clear()
            si.on_wait.append(waits[0])
            for w in waits[1:]:
                n2 = self.nc.sync.nop(nofuse=True)
                si2 = n2.ins.sync_info
                if si2 is None:
                    n2.ins.sync_info = si2 = mybir.SyncInfo(on_wait=[], on_update=[])
                si2.on_wait.append(w)
        self.nc.sync.drain()
        self.nc.all_engine_barrier()
        assert self.sems is not None
        popped = self.nc._tile_sem_poison_stack.pop()
        assert popped is self._sem_poison
        self.nc.clear_and_free_semaphores(list(self.sems.allocated().values()))
        self.nc.all_engine_barrier()



_TN = [0]


def _t(pool, shape, dtp, tag):
    _TN[0] += 1
    return pool.tile(shape, dtp, tag=tag, name=f"{tag}_{_TN[0]}")

def build_nc(cfg):
    nc = bacc.Bacc(None, target_bir_lowering=False, num_devices=NCORES)
    W, TW, EPW, shard = cfg.windows, cfg.tiles_w, cfg.epw, cfg.shard
    NS = cfg.nslice
    nsl = shard // NS
    rg = [list(range(NCORES))]

    def din(name, shape, dtp=dt.bfloat16):
        return nc.dram_tensor(name, shape, dtp, kind="ExternalInput")

    xT0 = din("xT0", [IN_DIM, shard])
    mask = din("mask", [1, shard])
    mask128 = din("mask128", [128, shard])
    gidx_d = din("gidx", [W, 128, EPW // 16], dt.int16)
    ind_d = din("ind", [W, 128, TW, 128])
    ea_d = din("ea_t", [17, cfg.epc])
    pool_d = din("pool_ind", [128, W, N_GRAPHS])
    invc_d = din("inv_cnt", [128, N_GRAPHS], dt.float32)
    ident_d = din("ident", [128, 128], dt.float32)
    wts = {}
    for li, (ci, c, cpad, has_ws) in enumerate(LAYERS):
        nk = (ci + 127) // 128
        if has_ws:
            wts[f"ws{li}"] = din(f"ws{li}", [128, nk, c])
            wts[f"bs{li}"] = din(f"bs{li}", [1, c])
        wts[f"we{li}"] = din(f"we{li}", [17, c])
        wts[f"w1_{li}"] = din(f"w1_{li}", [128, max(c // 128, 1), 2 * c])
        wts[f"b1_{li}"] = din(f"b1_{li}", [1, 2 * c])
        wts[f"w2_{li}"] = din(f"w2_{li}", [128, (2 * c) // 128, c])
        wts[f"b2_{li}"] = din(f"b2_{li}", [1, c])
        wts[f"g1_{li}"] = din(f"g1_{li}", [128, (2 * c) // 128], dt.float32)
        wts[f"be1_{li}"] = din(f"be1_{li}", [128, (2 * c) // 128], dt.float32)
    for nm, (ci, co) in [("dense1", (512, 512)), ("dense2", (512, 256)), ("dense3", (256, 10))]:
        wts[f"{nm}_w"] = din(f"{nm}_w", [128, ci // 128, co])
        wts[f"{nm}_b"] = din(f"{nm}_b", [1, co])

    out_d = nc.dram_tensor("out", [N_GRAPHS, OUT_DIM], dt.float32, kind="ExternalOutput")
    import os as _os3
    DBG = _os3.environ.get("GNN_DEBUG", "0") == "1"
    dbg_h = {}
    if DBG:
        for li, (ci, c, cpad, has_ws) in enumerate(LAYERS):
            dbg_h[li] = nc.dram_tensor(f"dbg_h{li}", [min(c, 128), max(c // 128, 1), shard],
                                       dt.bfloat16, kind="ExternalOutput")
            dbg_h[(li, "agg")] = nc.dram_tensor(
                f"dbg_agg{li}", [min(c, 128), max(c // 128, 1), shard],
                dt.bfloat16, kind="ExternalOutput")
            dbg_h[(li, "u")] = nc.dram_tensor(
                f"dbg_u{li}", [128, (2 * c) // 128, shard], dt.bfloat16, kind="ExternalOutput")
            dbg_h[(li, "st")] = nc.dram_tensor(
                f"dbg_st{li}", [128, ((2 * c) // 128) * 2], dt.float32, kind="ExternalOutput")

    ag_in, xt_full, st_in, st_out = {}, {}, {}, {}
    for li, (ci, c, cpad, has_ws) in enumerate(LAYERS):
        ag_in[li] = nc.dram_tensor(f"ag_in{li}", [shard, cpad], dt.bfloat16)
        xt_full[li] = nc.dram_tensor(f"xt_full{li}", [cfg.np_total, cpad], dt.bfloat16,
                                     addr_space="Shared")
        nst = ((2 * c) // 128) * 2
        st_in[li] = nc.dram_tensor(f"st_in{li}", [128, nst], dt.float32)
        st_out[li] = nc.dram_tensor(f"st_out{li}", [128, nst], dt.float32, addr_space="Shared")
    pool_in = nc.dram_tensor("pool_in", [128, 4 * N_GRAPHS], dt.float32)
    pool_out = nc.dram_tensor("pool_out", [128, 4 * N_GRAPHS], dt.float32, addr_space="Shared")

    inv_n = 1.0 / float(cfg.n_nodes)

    with TileContextP(nc) as tc:
        nc.gpsimd.load_library(_mlp_lib)
        with (
            tc.tile_pool(name="const", bufs=1) as cpool,
            tc.tile_pool(name="acts", bufs=1) as apool,
            tc.tile_pool(name="win", bufs=2) as wpool,
            tc.tile_pool(name="win1", bufs=1) as w1pool,
            tc.tile_pool(name="wt", bufs=1) as wtp,
            tc.tile_pool(name="small", bufs=2) as spool,
            tc.tile_pool(name="ps", bufs=2, space="PSUM") as psp,
            tc.tile_pool(name="ps1", bufs=1, space="PSUM") as ps1p,
        ):
            ident = _t(cpool, [128, 128], dt.float32, "ident")
            nc.sync.dma_start(ident[:], ident_d[:])
            mask_t = _t(cpool, [1, shard], dt.bfloat16, "mask")
            nc.sync.dma_start(mask_t[:], mask[:])
            mask128_t = _t(cpool, [128, shard], dt.bfloat16, "mask128")
            nc.sync.dma_start(mask128_t[:], mask128[:])
            invc_t = _t(cpool, [128, N_GRAPHS], dt.float32, "invc")
            nc.sync.dma_start(invc_t[:], invc_d[:])

            def new_xT(cdim, tag):
                nj_ = max(cdim // 128, 1)
                return _t(apool, [min(cdim, 128), nj_, shard], dt.bfloat16, tag)

            xT = new_xT(IN_DIM, "xT_a")
            nc.sync.dma_start(xT[:, 0, :], xT0[:])

            def matmul_chain(out_ps, w_tile, colsl, b_tile, rhs_tile, rhs_ci, n0, n1):
                """out_ps[M, n1-n0] = sum_k w[kchunk, cols].T @ rhs[kchunk, n0:n1]
                + b[cols].T @ mask[n0:n1]"""
                nj_ = max(rhs_ci // 128, 1)
                kc = min(rhs_ci, 128)
                c0, c1 = colsl
                for k in range(nj_):
                    nc.tensor.matmul(out_ps, w_tile[:kc, k, c0:c1], rhs_tile[:kc, k, n0:n1],
                                     start=(k == 0), stop=False)
                nc.tensor.matmul(out_ps, b_tile[:, c0:c1], mask_t[:, n0:n1],
                                 start=False, stop=True)

            for li, (ci, c, cpad, has_ws) in enumerate(LAYERS):
                nj = max(c // 128, 1)
                pdim = min(c, 128)
                nj2 = (2 * c) // 128
                nk_in = max(ci // 128, 1)

                # ---------- Phase A: xt = x @ Ws + bs (or alias)
                if has_ws:
                    w_ws = _t(wtp, [128, nk_in, c], dt.bfloat16, "wbig")
                    nc.sync.dma_start(w_ws[:], wts[f"ws{li}"][:])
                    b_ws = _t(wtp, [1, c], dt.bfloat16, "wb")
                    nc.sync.dma_start(b_ws[:], wts[f"bs{li}"][:])
                    xtT = _t(apool, [pdim, nj, shard], dt.bfloat16, "xtT")
                    for j in range(nj):
                        for n in range(nsl):
                            ps = _t(psp, [pdim, NS], dt.float32, "mmps")
                            matmul_chain(ps[:], w_ws, (j * 128, j * 128 + pdim), b_ws,
                                         xT, ci, n * NS, (n + 1) * NS)
                            nc.scalar.copy(xtT[:, j, n * NS:(n + 1) * NS], ps[:])
                else:
                    xtT = xT

                # ---------- Phase A': row-major bf16 + AllGather
                xt_row = _t(w1pool, [128, W, cpad], dt.bfloat16, "msg")
                if cpad != c:
                    nc.vector.memset(xt_row[:, :, c:cpad], 0.0)
                for j in range(nj):
                    for t in range(W):
                        nc.sync.dma_start_transpose(
                            xt_row[:, t, j * 128:j * 128 + pdim],
                            xtT[:pdim, j, t * 128:(t + 1) * 128],
                        )
                nc.sync.dma_start(
                    ag_in[li][:].rearrange("(t p) c -> p t c", p=128),
                    xt_row[:],
                )
                nc.gpsimd.collective_compute(
                    "AllGather", ALU.bypass, replica_groups=rg,
                    ins=[ag_in[li][:].opt()], outs=[xt_full[li][:].opt()],
                )

                # ---------- Phase B: edge stage
                w_we = _t(wtp, [17, c], dt.bfloat16, "wwe")
                nc.sync.dma_start(w_we[:], wts[f"we{li}"][:])
                maxct = max(cfg.chunks)
                for w in range(W):
                    seg_e = _t(ps1p, [128, c], dt.float32, "sege")
                    seg_p = _t(ps1p, [128, c], dt.float32, "segp")
                    gt = 0
                    colbase = 0
                    maxct = max(cfg.chunks)
                    for ct in cfg.chunks:
                        nidx = ct * 128
                        gx = _t(wpool, [128, maxct, cpad], dt.bfloat16, "gx")
                        gi = _t(wpool, [128, EPW // 16], dt.int16, "gi")
                        nc.sync.dma_start(gi[:, colbase:colbase + nidx // 16],
                                          gidx_d[w, :, colbase:colbase + nidx // 16])
                        nc.gpsimd.dma_gather(
                            gx[:, :ct, :], xt_full[li][:],
                            gi[:, colbase:colbase + nidx // 16],
                            nidx, nidx, cpad,
                        )
                        ea_w = _t(wpool, [17, maxct * 128], dt.bfloat16, "eaw")
                        nc.sync.dma_start(
                            ea_w[:, :nidx],
                            ea_d[:, w * EPW + gt * 128: w * EPW + gt * 128 + nidx],
                        )
                        msg = _t(wpool, [128, maxct, c], dt.bfloat16, "msgb")
                        ext = _t(wpool, [128, maxct, c], dt.bfloat16, "extb")
                        for t in range(ct):
                            eps = _t(psp, [128, c], dt.float32, "eps")
                            nc.tensor.matmul(eps[:], ea_w[:, t * 128:(t + 1) * 128],
                                             w_we[:], start=True, stop=True)
                            nc.vector.tensor_add(msg[:, t, :], gx[:, t, :c], eps[:])
                        nc.vector.tensor_scalar_max(msg[:, :ct, :], msg[:, :ct, :], 0.0)
                        nc.scalar.activation(ext[:, :ct, :], msg[:, :ct, :], AF.Exp)
                        nc.vector.tensor_mul(msg[:, :ct, :], msg[:, :ct, :], ext[:, :ct, :])
                        ind_w = _t(wpool, [128, maxct, 128], dt.bfloat16, "indw")
                        nc.sync.dma_start(ind_w[:, :ct, :], ind_d[w, :, gt:gt + ct, :])
                        for t in range(ct):
                            for hh in range(0, c, 512):
                                he = min(hh + 512, c)
                                nc.tensor.matmul(
                                    seg_e[:, hh:he], ind_w[:, t, :], ext[:, t, hh:he],
                                    start=(gt + t == 0), stop=(gt + t == TW - 1),
                                    skip_group_check=(hh > 0),
                                )
                                nc.tensor.matmul(
                                    seg_p[:, hh:he], ind_w[:, t, :], msg[:, t, hh:he],
                                    start=(gt + t == 0), stop=(gt + t == TW - 1),
                                    skip_group_check=(hh > 0),
                                )
                        gt += ct
                        colbase += nidx // 16
                    dwin = _t(spool, [128, c], dt.float32, "dwin")
                    nc.vector.tensor_scalar_add(dwin[:], seg_e[:], EPS_DEN)
                    rec = _t(spool, [128, c], dt.float32, "rec")
                    nc.vector.reciprocal_approx_fast(rec[:], dwin[:])
                    aggr = _t(spool, [128, c], dt.float32, "aggr")
                    nc.vector.tensor_mul(aggr[:], rec[:], seg_p[:])
                    for j in range(nj):
                        tps = _t(psp, [128, 128], dt.float32, "trps")
                        nc.tensor.transpose(tps[:pdim, :], aggr[:, j * 128:j * 128 + pdim],
                                            ident[:])
                        nc.vector.tensor_add(
                            xtT[:pdim, j, w * 128:(w + 1) * 128],
                            xtT[:pdim, j, w * 128:(w + 1) * 128],
                            tps[:pdim, :],
                        )

                if DBG:
                    nc.sync.dma_start(dbg_h[(li, "agg")][:], xtT[:pdim, :nj, :])

                # ---------- Phase C: u = h_mid @ W1 + b1; BN; relu; W2
                w_w1 = _t(wtp, [128, nj, 2 * c], dt.bfloat16, "wbig")
                nc.sync.dma_start(w_w1[:], wts[f"w1_{li}"][:])
                b_w1 = _t(wtp, [1, 2 * c], dt.bfloat16, "wb")
                nc.sync.dma_start(b_w1[:], wts[f"b1_{li}"][:])
                u = _t(apool, [128, nj2, shard], dt.bfloat16, "u")
                statsS = _t(spool, [128, nj2, nsl], dt.float32, "statsS")
                stats2 = _t(spool, [128, nj2, nsl], dt.float32, "stats2")
                junk = _t(apool, [128, NS], dt.bfloat16, "junk")
                for j2 in range(nj2):
                    for n in range(nsl):
                        ps = _t(psp, [128, NS], dt.float32, "mmps")
                        matmul_chain(ps[:], w_w1, (j2 * 128, (j2 + 1) * 128), b_w1,
                                     xtT, c, n * NS, (n + 1) * NS)
                        nc.scalar.copy(u[:, j2, n * NS:(n + 1) * NS], ps[:])
                        nc.vector.reduce_sum(statsS[:, j2, n:n + 1], ps[:],
                                             axis=mybir.AxisListType.X)
                        nc.scalar.activation(junk[:], ps[:], AF.Square,
                                             accum_out=stats2[:, j2, n:n + 1])
                stats = _t(spool, [128, nj2, 2], dt.float32, "stats")
                nc.vector.reduce_sum(stats[:, :, 0:1], statsS[:],
                                     axis=mybir.AxisListType.X)
                nc.vector.reduce_sum(stats[:, :, 1:2], stats2[:],
                                     axis=mybir.AxisListType.X)
                if DBG:
                    nc.sync.dma_start(dbg_h[(li, "u")][:], u[:, :nj2, :])
                    nc.sync.dma_start(dbg_h[(li, "st")][:], stats[:].rearrange("p a b -> p (a b)"))
                nc.sync.dma_start(st_in[li][:], stats[:].rearrange("p a b -> p (a b)"))
                nc.gpsimd.collective_compute(
                    "AllReduce", ALU.add, replica_groups=rg,
                    ins=[st_in[li][:].opt()], outs=[st_out[li][:].opt()],
                )
                statr = _t(spool, [128, nj2, 2], dt.float32, "statr")
                nc.sync.dma_start(statr[:].rearrange("p a b -> p (a b)"), st_out[li][:])
                g1t = _t(spool, [128, nj2], dt.float32, "g1t")
                nc.sync.dma_start(g1t[:], wts[f"g1_{li}"][:])
                be1t = _t(spool, [128, nj2], dt.float32, "be1t")
                nc.sync.dma_start(be1t[:], wts[f"be1_{li}"][:])
                mu = _t(spool, [128, nj2], dt.float32, "mu")
                nc.vector.tensor_scalar_mul(mu[:], statr[:, :, 0], inv_n)
                msq = _t(spool, [128, nj2], dt.float32, "msq")
                nc.vector.tensor_scalar_mul(msq[:], statr[:, :, 1], inv_n)
                var = _t(spool, [128, nj2], dt.float32, "var")
                nc.vector.scalar_tensor_tensor(var[:], mu[:], -1.0, mu[:],
                                               op0=ALU.mult, op1=ALU.mult)
                nc.vector.tensor_add(var[:], var[:], msq[:])
                nc.vector.tensor_scalar_max(var[:], var[:], 0.0)
                vr = _t(spool, [128, nj2], dt.float32, "vr")
                nc.vector.tensor_scalar_add(vr[:], var[:], BN_EPS)
                rvr = _t(spool, [128, nj2], dt.float32, "rvr")
                nc.vector.reciprocal(rvr[:], vr[:])
                inv_std = _t(spool, [128, nj2], dt.float32, "invs")
                nc.scalar.sqrt(inv_std[:], rvr[:])
                A_t = _t(spool, [128, nj2], dt.float32, "A_t")
                nc.vector.tensor_mul(A_t[:], g1t[:], inv_std[:])
                B_t = _t(spool, [128, nj2], dt.float32, "B_t")
                nc.vector.scalar_tensor_tensor(B_t[:], mu[:], -1.0, A_t[:],
                                               op0=ALU.mult, op1=ALU.mult)
                nc.vector.tensor_add(B_t[:], B_t[:], be1t[:])
                for j2 in range(nj2):
                    nc.scalar.activation(u[:, j2, :], u[:, j2, :], AF.Relu,
                                         bias=B_t[:, j2:j2 + 1], scale=A_t[:, j2:j2 + 1])
                    nc.vector.tensor_mul(u[:, j2, :], u[:, j2, :], mask128_t[:])
                w_w2 = _t(wtp, [128, nj2, c], dt.bfloat16, "wbig")
                nc.sync.dma_start(w_w2[:], wts[f"w2_{li}"][:])
                b_w2 = _t(wtp, [1, c], dt.bfloat16, "wb")
                nc.sync.dma_start(b_w2[:], wts[f"b2_{li}"][:])
                xT_next = new_xT(c, "xT_b" if li % 2 == 0 else "xT_a")
                for j in range(nj):
                    for n in range(nsl):
                        ps = _t(psp, [pdim, NS], dt.float32, "mmps")
                        matmul_chain(ps[:], w_w2, (j * 128, j * 128 + pdim), b_w2,
                                     u, 2 * c, n * NS, (n + 1) * NS)
                        nc.scalar.activation(xT_next[:, j, n * NS:(n + 1) * NS],
                                             ps[:], AF.Relu)
                if DBG:
                    nc.sync.dma_start(dbg_h[li][:], xT_next[:pdim, :nj, :])
                xT = xT_next

            # ---------- pooling + head
            h_row = _t(w1pool, [128, W, 512], dt.bfloat16, "msg")
            for j in range(4):
                for t in range(W):
                    nc.sync.dma_start_transpose(
                        h_row[:, t, j * 128:(j + 1) * 128],
                        xT[:, j, t * 128:(t + 1) * 128],
                    )
            pind_t = _t(cpool, [128, W, N_GRAPHS], dt.bfloat16, "pind")
            nc.sync.dma_start(pind_t[:], pool_d[:])
            gsb = _t(spool, [128, 4, N_GRAPHS], dt.float32, "gsb")
            for j in range(4):
                gps = _t(ps1p, [128, N_GRAPHS], dt.float32, "sege")
                for t in range(W):
                    nc.tensor.matmul(gps[:], h_row[:, t, j * 128:(j + 1) * 128],
                                     pind_t[:, t, :], start=(t == 0), stop=(t == W - 1))
                nc.vector.tensor_copy(gsb[:, j, :], gps[:])
            nc.sync.dma_start(pool_in[:], gsb[:].rearrange("p a b -> p (a b)"))
            nc.gpsimd.collective_compute(
                "AllReduce", ALU.add, replica_groups=rg,
                ins=[pool_in[:].opt()], outs=[pool_out[:].opt()],
            )
            gT = _t(spool, [128, 4, N_GRAPHS], dt.float32, "gT")
            nc.sync.dma_start(gT[:].rearrange("p a b -> p (a b)"), pool_out[:])
            gTb = _t(spool, [128, 4, N_GRAPHS], dt.bfloat16, "gTb")
            for j in range(4):
                nc.vector.tensor_mul(gTb[:, j, :], gT[:, j, :], invc_t[:])

            ones50 = mask_t[:, :N_GRAPHS]

            def dense_T(src_t, srcchunks, wname, co):
                wt_ = _t(wtp, [128, srcchunks, co], dt.bfloat16, "wbig")
                nc.sync.dma_start(wt_[:], wts[f"{wname}_w"][:])
                bt_ = _t(wtp, [1, co], dt.bfloat16, "wb")
                nc.sync.dma_start(bt_[:], wts[f"{wname}_b"][:])
                out_t = _t(spool, [128, co // 128, N_GRAPHS], dt.bfloat16, f"dT{co}")
                for j in range(co // 128):
                    ps = _t(psp, [128, N_GRAPHS], dt.float32, "mmps")
                    for k in range(srcchunks):
                        nc.tensor.matmul(ps[:], wt_[:, k, j * 128:(j + 1) * 128],
                                         src_t[:, k, :], start=(k == 0), stop=False)
                    nc.tensor.matmul(ps[:], bt_[:, j * 128:(j + 1) * 128],
                                     ones50, start=False, stop=True)
                    nc.vector.tensor_copy(out_t[:, j, :], ps[:])
                return out_t

            g1_t = dense_T(gTb, 4, "dense1", 512)
            g2_t = dense_T(g1_t, 4, "dense2", 256)
            w3t = _t(wtp, [128, 2, OUT_DIM], dt.bfloat16, "wbig")
            nc.sync.dma_start(w3t[:], wts["dense3_w"][:])
            b3t = _t(wtp, [1, OUT_DIM], dt.bfloat16, "wb")
            nc.sync.dma_start(b3t[:], wts["dense3_b"][:])
            o3 = _t(ps1p, [N_GRAPHS, OUT_DIM], dt.float32, "sege")
            for k in range(2):
                nc.tensor.matmul(o3[:], g2_t[:, k, :], w3t[:, k, :],
                                 start=(k == 0), stop=False)
            nc.tensor.matmul(o3[:], ones50, b3t[:], start=False, stop=True)
            mx = _t(spool, [N_GRAPHS, 1], dt.float32, "mx")
            nc.vector.reduce_max(mx[:], o3[:], axis=mybir.AxisListType.X)
            nmx = _t(spool, [N_GRAPHS, 1], dt.float32, "nmx")
            nc.vector.tensor_scalar_mul(nmx[:], mx[:], -1.0)
            ex3 = _t(spool, [N_GRAPHS, OUT_DIM], dt.float32, "ex3")
            nc.scalar.activation(ex3[:], o3[:], AF.Exp, bias=nmx[:])
            s3 = _t(spool, [N_GRAPHS, 1], dt.float32, "s3")
            nc.vector.reduce_sum(s3[:], ex3[:], axis=mybir.AxisListType.X)
            ls3 = _t(spool, [N_GRAPHS, 1], dt.float32, "ls3")
            nc.scalar.activation(ls3[:], s3[:], AF.Ln)
            off = _t(spool, [N_GRAPHS, 1], dt.float32, "off")
            nc.vector.scalar_tensor_tensor(off[:], ls3[:], -1.0, nmx[:],
                                           op0=ALU.mult, op1=ALU.add)
            fin = _t(spool, [N_GRAPHS, OUT_DIM], dt.float32, "fin")
            nc.scalar.activation(fin[:], o3[:], AF.Identity, bias=off[:])
            nc.sync.dma_start(out_d[:], fin[:])

    nc.compile()
    return nc


_CACHE = {}


def _get_nc(cfg):
    key = (cfg.n_nodes, cfg.shard, cfg.tiles_w)
    if key not in _CACHE:
        _CACHE[key] = build_nc(cfg)
    return _CACHE[key]


def run(cfg, x, edge_attr, params, edge_index, batch, trace=False):
    in_maps = host_prep(cfg, x, edge_attr, params, edge_index, batch)
    nc = _get_nc(cfg)
    return run_bass_kernel_spmd(nc, in_maps, core_ids=list(range(NCORES)), trace=trace)


def kernel(x, edge_attr, params, edge_index, batch):
    res = run(FULL, x, edge_attr, params, edge_index, batch)
    return res.results[0]["out"]
